# revision 1
# baseline (speedup 1.0000x reference)
"""GQA causal prefill attention on 8 TRN2 NeuronCores.

Sharding: head-parallel. Core c computes q heads [4c, 4c+4) against kv head c
(n_rep = 4, so the GQA groups align exactly with the shard; no cross-core
communication).

Per-core algorithm (T=2048 tokens, 4 q heads, head_dim 128):
  - Load k, v; build kT (d,s) tiles via PE transpose. v is augmented with a
    ones column -> v_aug (s, 129) in bf16.
  - Per head h: build qT (d,t) via PE transpose; for each s-tile j compute
    S^T_j = k_j @ q_h^T (s=128 partitions, t>=j*128 free) on PE (bf16,
    f32 PSUM), exp(scale*S^T) on ScalarE straight from PSUM into bf16 SBUF
    (causal diagonal tile masked by an upper-triangular multiply).
  - PV with the e^T blocks as the stationary operand and v_aug streaming:
    out_psum (t=128, 129) accumulates over j; column 128 is the softmax
    denominator. Normalize with a per-partition reciprocal multiply and DMA
    the (t, d) tile to DRAM.
"""

import sys
import functools

import numpy as np

if "/opt/trn_rl_repo" not in sys.path:
    sys.path.insert(0, "/opt/trn_rl_repo")

T = 2048
H_TOTAL = 32
N_CORES = 8
H = H_TOTAL // N_CORES  # 4 q heads per core
D = 128
P = 128
NT = T // P  # 16 token tiles
SCALE = 0.08838834764831845

# column offset of s-tile j's slice inside the per-head packed e^T buffer
_EOFF = [0] * (NT + 1)
for _j in range(NT):
    _EOFF[_j + 1] = _EOFF[_j] + (T - P * _j)
E_COLS = _EOFF[NT]  # 17408


def _n_chunks(n_tiles):
    """Split n_tiles 128-col tiles into matmul chunks of <=4 tiles (<=512 cols)."""
    out = []
    i = 0
    while i < n_tiles:
        c = min(4, n_tiles - i)
        out.append((i, c))
        i += c
    return out


def _build_body(tc, nc, q_d, k_d, v_d, o_d, ctx):
    from collections import deque

    import concourse.mybir as mybir
    from concourse.masks import make_identity, make_upper_triangular

    f32 = mybir.dt.float32
    bf16 = mybir.dt.bfloat16

    const = ctx.enter_context(tc.tile_pool(name="const", bufs=1))
    qbp = ctx.enter_context(tc.tile_pool(name="qbf", bufs=4))
    qtp = ctx.enter_context(tc.tile_pool(name="qT", bufs=4))
    ep = ctx.enter_context(tc.tile_pool(name="eT", bufs=2))
    outp = ctx.enter_context(tc.tile_pool(name="outt", bufs=4))
    recp = ctx.enter_context(tc.tile_pool(name="rec", bufs=4))

    # PSUM: two 3-bank S^T units (ping-pong) + two shared 1-bank slots for
    # PV accumulators and transpose staging = exactly 8 banks.
    st_pool = ctx.enter_context(tc.tile_pool(name="st", bufs=2, space="PSUM"))
    sm_pool = ctx.enter_context(tc.tile_pool(name="smp", bufs=2, space="PSUM"))

    identity = const.tile([P, P], bf16, tag="ident")
    make_identity(nc, identity)
    utri = const.tile([P, P], bf16, tag="utri")
    make_upper_triangular(nc, utri, val=1.0, diag=True)

    q_view = q_d.rearrange("(i p) h d -> p i h d", p=P)
    o_view = o_d.rearrange("(i p) h d -> p i h d", p=P)
    NB = NT // 4  # 4-tile batches

    # Prewarm the ACT function table so the first real exp doesn't pay the
    # ~1.3us table load on the critical path.
    warm_sb = recp.tile([P, 1], f32, tag="rec", name="warm")
    nc.scalar.activation(
        out=warm_sb, in_=identity[:, 0:1],
        func=mybir.ActivationFunctionType.Exp,
    )

    # DMA (all on the SP ring, FIFO): reverse-chunked k and q0 — head 0 runs
    # its j-loop descending, so the LAST chunks are needed first and compute
    # starts after only ~0.5MB has landed. v and q1-3 follow.
    k_view = k_d.rearrange("(j p) d -> p j d", p=P)
    k_sb = const.tile([P, NT, D], f32, tag="ksb")
    q0_sb = qbp.tile([P, NT, D], f32, tag="qstg", name="q0stg")
    CHUNK_ORDER = [3, 2, 0, 1]  # matches head 0's j order: 15..8 then 0..7
    for b in CHUNK_ORDER:
        nc.sync.dma_start(k_sb[:, 4 * b:4 * b + 4, :], k_view[:, 4 * b:4 * b + 4, :])
        nc.sync.dma_start(q0_sb[:, 4 * b:4 * b + 4, :], q_view[:, 4 * b:4 * b + 4, 0, :])
    v_sb = const.tile([P, NT, D], f32, tag="vsb")
    v_view = v_d.rearrange("(j p) d -> p j d", p=P)
    for b in range(2):
        nc.sync.dma_start(v_sb[:, 8 * b:8 * b + 8, :], v_view[:, 8 * b:8 * b + 8, :])
    q_stg = [None] * H
    q_stg[0] = q0_sb
    for h in range(1, H):
        stg = qbp.tile([P, NT, D], f32, tag="qstg", name=f"q{h}stg")
        nc.sync.dma_start(stg, q_view[:, :, h, :])
        q_stg[h] = stg

    # HAM pre-warm: cheap dummy transposes keep the PE busy during the DMA
    # wait so the clock gate is at 8/8 when real work starts.
    warm_ps = sm_pool.tile([P, 4 * P], bf16, tag="sm", name="warmps")
    for _ in range(32):
        nc.tensor.transpose(warm_ps[0:64, 0:P], identity[:, 0:64], identity)

    def transpose_batch(dst, src, b):
        """Transpose 4 (128,128) bf16 tiles src[:, 4b+m, :] into dst[:, 4b+m, :]
        through one 1-bank PSUM tile and a single batched copy."""
        tp = sm_pool.tile([P, 4 * P], bf16, tag="sm")
        for m in range(4):
            nc.tensor.transpose(tp[:, m * P:(m + 1) * P], src[:, 4 * b + m, :], identity)
        nc.vector.tensor_copy(out=dst[:, 4 * b:4 * b + 4, :], in_=tp)

    q_bf = [None] * H

    def ensure_qbf(h):
        if q_bf[h] is None:
            qb = qbp.tile([P, NT, D], bf16, tag="qbf", name=f"qbf{h}")
            nc.vector.tensor_copy(out=qb, in_=q_stg[h])
            q_bf[h] = qb
        return q_bf[h]

    # Interleave k/q0 chunk casts and transpose batches in DMA-arrival
    # (reverse-chunk) order.
    k_bf = const.tile([P, NT, D], bf16, tag="kbf")
    kT = const.tile([P, NT, P], bf16, tag="kT")  # [d, j, s]
    qb0 = qbp.tile([P, NT, D], bf16, tag="qbf", name="qbf0")
    q_bf[0] = qb0
    qT = [
        qtp.tile([P, NT, P], bf16, tag="qT", name=f"qT{h}") for h in range(H)
    ]  # [d, i, t]
    for b in CHUNK_ORDER:
        nc.vector.tensor_copy(
            out=k_bf[:, 4 * b:4 * b + 4, :], in_=k_sb[:, 4 * b:4 * b + 4, :])
        transpose_batch(kT, k_bf, b)
        nc.vector.tensor_copy(
            out=qb0[:, 4 * b:4 * b + 4, :], in_=q0_sb[:, 4 * b:4 * b + 4, :])
        transpose_batch(qT[0], qb0, b)

    v_aug = const.tile([P, NT, D + 1], bf16, tag="vaug")
    nc.vector.tensor_copy(out=v_aug[:, :, 0:D], in_=v_sb)
    nc.vector.memset(v_aug[:, :, D:D + 1], 1.0)

    # filler thunks: (head, batch) transposes for heads 1..3
    fillers = deque(
        (h, b) for h in range(1, H) for b in range(NT // 4)
    )

    def emit_filler():
        fh, fb = fillers.popleft()
        transpose_batch(qT[fh], ensure_qbf(fh), fb)

    def emit_fillers_for_head(h):
        while fillers and fillers[0][0] <= h:
            emit_filler()

    def emit_chain(eT, h, i):
        """PV accumulation for t-tile i of head h: out_psum (t,129); col 128 is
        the softmax denominator. Normalize and DMA out."""
        pv = sm_pool.tile([P, P + 1], f32, tag="sm")
        for j in range(i + 1):
            c0 = _EOFF[j] + (i - j) * P
            nc.tensor.matmul(
                pv,
                lhsT=eT[:, c0:c0 + P],
                rhs=v_aug[:, j, :],
                start=(j == 0),
                stop=(j == i),
            )
        rec = recp.tile([P, 1], f32, tag="rec")
        nc.vector.reciprocal(rec, pv[:, D:D + 1])
        ot = outp.tile([P, D], f32, tag="outt")
        nc.vector.tensor_scalar_mul(ot, pv[:, 0:D], rec)
        nc.sync.dma_start(o_view[:, i, h, :], ot)

    ready = deque()  # (eT, head, i) PV chains not yet emitted

    def pop_ready(budget, force=False):
        while ready:
            e2, h2, i2 = ready[0]
            size = i2 + 1
            if not force and size > budget and budget < 16:
                break
            ready.popleft()
            emit_chain(e2, h2, i2)
            budget -= size
            if budget <= 0 and not force:
                break

    ST_TILES = 12  # 1536 cols = 3 PSUM banks per S^T unit

    for h in range(H):
        eT = ep.tile([P, E_COLS], bf16, tag="eT")
        # Head 0 walks j=15..8 (so the first step needs only the last k/q DMA
        # chunk and compute starts ~15us earlier), then 0..7 ascending so its
        # own small PV chains become ready to fill the late steps.
        j_order = (
            list(range(NT - 1, 7, -1)) + list(range(0, 8)) if h == 0 else range(NT)
        )
        for j in j_order:
            # PE work for the exp window FIRST: in-order engine streams mean
            # anything emitted after S^T(j)'s psum-wait would be stuck
            # behind it.
            pop_ready((NT - j) + (8 if h == H - 1 else 2))
            if fillers and (
                (h == 0 and j < 8)
                or (0 < h < H - 1 and fillers[0][0] == h + 1 and j % 2 == 0)
            ):
                emit_filler()
            ntiles = NT - j
            off = _EOFF[j]
            if ntiles > ST_TILES:
                g0 = (ntiles + 1) // 2
                groups = [(0, g0), (g0, ntiles - g0)]
            else:
                groups = [(0, ntiles)]
            for (gb, gn) in groups:
                stu = st_pool.tile([P, ST_TILES * P], f32, tag="st")
                for (i0, ci) in _n_chunks(gn):
                    nc.tensor.matmul(
                        stu[:, i0 * P:(i0 + ci) * P],
                        lhsT=kT[:, j, :],
                        rhs=qT[h][:, j + gb + i0:j + gb + i0 + ci, :],
                        start=True,
                        stop=True,
                    )
                nc.scalar.activation(
                    out=eT[:, off + gb * P:off + (gb + gn) * P],
                    in_=stu[:, 0:gn * P],
                    func=mybir.ActivationFunctionType.Exp,
                    scale=SCALE,
                )
            # causal mask on the diagonal tile: keep t_local >= s_local
            nc.vector.tensor_tensor(
                eT[:, off:off + P],
                eT[:, off:off + P],
                utri,
                mybir.AluOpType.mult,
            )
            if h > 0:
                ready.append((eT, h, j))
            elif j < 8:
                # ascending tail of head 0: exps 8..15 and 0..j are all done,
                # so chain j is complete
                ready.append((eT, 0, j))
        if h == 0:
            for i in range(8, NT):
                ready.append((eT, 0, i))
        if h + 1 < H:
            emit_fillers_for_head(h + 1)
        if h >= 1:
            # everything from head h-1 must drain before its eT slot recycles
            while ready and ready[0][1] < h:
                e2, h2, i2 = ready.popleft()
                emit_chain(e2, h2, i2)
    pop_ready(0, force=True)


@functools.lru_cache(maxsize=1)
def _build():
    import concourse.tile as tile
    import concourse.mybir as mybir
    from concourse import bacc
    from contextlib import ExitStack

    f32 = mybir.dt.float32
    nc = bacc.Bacc(
        "TRN2",
        target_bir_lowering=False,
        debug=False,
        num_devices=N_CORES,
    )
    q_d = nc.dram_tensor("q", (T, H, D), f32, kind="ExternalInput").ap()
    k_d = nc.dram_tensor("k", (T, D), f32, kind="ExternalInput").ap()
    v_d = nc.dram_tensor("v", (T, D), f32, kind="ExternalInput").ap()
    o_d = nc.dram_tensor("out", (T, H, D), f32, kind="ExternalOutput").ap()

    with tile.TileContext(nc) as tc:
        with ExitStack() as ctx:
            _build_body(tc, nc, q_d, k_d, v_d, o_d, ctx)
    nc.compile()
    return nc


def _in_maps(q, k, v):
    q = np.asarray(q, dtype=np.float32)
    k = np.asarray(k, dtype=np.float32)
    v = np.asarray(v, dtype=np.float32)
    return [
        {
            "q": np.ascontiguousarray(q[:, H * c:H * c + H, :]),
            "k": np.ascontiguousarray(k[:, c, :]),
            "v": np.ascontiguousarray(v[:, c, :]),
        }
        for c in range(N_CORES)
    ]


def kernel(q, k, v, _trace=False):
    from concourse.bass_utils import run_bass_kernel_spmd

    nc = _build()
    res = run_bass_kernel_spmd(
        nc, _in_maps(q, k, v), core_ids=list(range(N_CORES)), trace=_trace
    )
    out = np.empty((T, H_TOTAL, D), dtype=np.float32)
    for c in range(N_CORES):
        out[:, H * c:H * c + H, :] = res.results[c]["out"].reshape(T, H, D)
    if _trace:
        return out, res
    return out



# revision 3
# speedup vs baseline: 1.0184x; 1.0184x over previous
"""GQA causal prefill attention on 8 TRN2 NeuronCores.

Sharding: head-parallel. Core c computes q heads [4c, 4c+4) against kv head c
(n_rep = 4, so the GQA groups align exactly with the shard; no cross-core
communication).

Per-core algorithm (T=2048 tokens, 4 q heads, head_dim 128):
  - Load k, v; build kT (d,s) tiles via PE transpose. v is augmented with a
    ones column -> v_aug (s, 129) in bf16.
  - S^T tiles are packed into 12-tile (3 PSUM bank) units spanning several
    k-tile rows j, so ONE ScalarE exp instruction covers a whole unit. The
    ACT engine is the binding resource (~1.0 ns/col + ~290 ns/inst); packing
    minimizes the per-instruction overhead (~50 exps per core).
  - Causal diagonal masking is done in-place on the bf16 eT buffer by GpSimd
    affine_select (t_local >= s_local), keeping both DVE and ACT off that
    path. q1-3 and v f32->bf16 casts also run on GpSimd.
  - PV with the e^T blocks as the stationary operand and v_aug streaming:
    out_psum (t=128, 129) accumulates over j; column 128 is the softmax
    denominator. Normalize with a per-partition reciprocal multiply (DVE) and
    DMA the (t, d) tile to DRAM.
"""

import sys
import functools

import numpy as np

if "/opt/trn_rl_repo" not in sys.path:
    sys.path.insert(0, "/opt/trn_rl_repo")

T = 2048
H_TOTAL = 32
N_CORES = 8
H = H_TOTAL // N_CORES  # 4 q heads per core
D = 128
P = 128
NT = T // P  # 16 token tiles
SCALE = 0.08838834764831845
UNIT = 12  # tiles per exp unit = 3 PSUM banks

# column offset of s-tile j's slice inside the per-head packed e^T buffer
_EOFF = [0] * (NT + 1)
for _j in range(NT):
    _EOFF[_j + 1] = _EOFF[_j] + (T - P * _j)
E_COLS = _EOFF[NT]  # 17408


def _split_asc(blocks, tail=None):
    """Chop an ascending stream of (j, ilo, ntiles) blocks into units of
    <= UNIT tiles, splitting blocks at tile granularity. `tail` optionally
    forces the sizes of the final units (e.g. [3, 1] for a short drain)."""
    sizes = []
    total = sum(b[2] for b in blocks)
    if tail:
        head = total - sum(tail)
        sizes = [UNIT] * (head // UNIT)
        if head % UNIT:
            sizes.append(head % UNIT)
        sizes += tail
    else:
        sizes = [UNIT] * (total // UNIT)
        if total % UNIT:
            sizes.append(total % UNIT)
    units = []
    cur = []
    cur_n = 0
    si = 0
    for (j, ilo, ln) in blocks:
        a = 0
        while a < ln:
            take = min(sizes[si] - cur_n, ln - a)
            cur.append((j, ilo + a, take))
            cur_n += take
            a += take
            if cur_n == sizes[si]:
                units.append(cur)
                cur = []
                cur_n = 0
                si += 1
    assert not cur
    return units


def _head_units(h):
    """Per head: list of units; unit = list of pieces (j, ilo, ntiles).
    Head 0 starts with descending block-aligned groups (j=15..8) so compute
    begins after only the tail k/q DMA chunks have landed; then j=0..7
    ascending. Heads 1-3 (and head 3 with a short tail) run j ascending."""
    if h == 0:
        units = [
            [(j, j, NT - j) for j in (15, 14, 13, 12)],  # 10 tiles
            [(j, j, NT - j) for j in (11, 10)],  # 11 tiles
            [(9, 9, 7)],
            [(8, 8, 8)],
        ]
        units += _split_asc([(j, j, NT - j) for j in range(8)])
        return units
    tail = [3, 1] if h == H - 1 else None
    return _split_asc([(j, j, NT - j) for j in range(NT)], tail=tail)


def _build_body(tc, nc, q_d, k_d, v_d, o_d, ctx):
    from collections import deque

    import concourse.mybir as mybir
    from concourse.masks import make_identity

    f32 = mybir.dt.float32
    bf16 = mybir.dt.bfloat16

    const = ctx.enter_context(tc.tile_pool(name="const", bufs=1))
    qbp = ctx.enter_context(tc.tile_pool(name="qbf", bufs=4))
    qtp = ctx.enter_context(tc.tile_pool(name="qT", bufs=4))
    ep = ctx.enter_context(tc.tile_pool(name="eT", bufs=2))
    outp = ctx.enter_context(tc.tile_pool(name="outt", bufs=4))
    recp = ctx.enter_context(tc.tile_pool(name="rec", bufs=4))

    # PSUM: two 3-bank S^T units (ping-pong) + two shared 1-bank slots for
    # PV accumulators and transpose staging = exactly 8 banks.
    st_pool = ctx.enter_context(tc.tile_pool(name="st", bufs=2, space="PSUM"))
    sm_pool = ctx.enter_context(tc.tile_pool(name="smp", bufs=2, space="PSUM"))

    # DMA (all on the SP ring, FIFO). k/q0 chunk order matches the compute
    # order: head 0 walks j=15..8 first (needs k/q tail chunks), then 0..7.
    k_view = k_d.rearrange("(j p) d -> p j d", p=P)
    q_view = q_d.rearrange("(i p) h d -> p i h d", p=P)
    o_view = o_d.rearrange("(i p) h d -> p i h d", p=P)
    v_view = v_d.rearrange("(j p) d -> p j d", p=P)

    k_sb = const.tile([P, NT, D], f32, tag="ksb")
    q0_sb = qbp.tile([P, NT, D], f32, tag="qstg", name="q0stg")
    v_sb = const.tile([P, NT, D], f32, tag="vsb")

    def dma_k(b):
        nc.sync.dma_start(k_sb[:, 4 * b:4 * b + 4, :], k_view[:, 4 * b:4 * b + 4, :])

    def dma_q0(b):
        nc.sync.dma_start(
            q0_sb[:, 4 * b:4 * b + 4, :], q_view[:, 4 * b:4 * b + 4, 0, :])

    dma_k(3); dma_q0(3)
    dma_k(2); dma_q0(2)
    dma_k(0); dma_q0(0)
    dma_q0(1); dma_k(1)
    for b in range(2):
        nc.sync.dma_start(v_sb[:, 8 * b:8 * b + 8, :], v_view[:, 8 * b:8 * b + 8, :])
    q_stg = [None] * H
    q_stg[0] = q0_sb
    for h in range(1, H):
        stg = qbp.tile([P, NT, D], f32, tag="qstg", name=f"q{h}stg")
        nc.sync.dma_start(stg, q_view[:, :, h, :])
        q_stg[h] = stg

    identity = const.tile([P, P], bf16, tag="ident")
    make_identity(nc, identity)
    zfill = nc.gpsimd.to_reg(0.0)

    # Prewarm the ACT function table so the first real exp doesn't pay the
    # ~1.5us table load on the critical path.
    warm_sb = recp.tile([P, 1], f32, tag="rec", name="warm")
    nc.scalar.activation(
        out=warm_sb, in_=identity[:, 0:1],
        func=mybir.ActivationFunctionType.Exp,
    )

    # Short HAM pre-warm: dummy transposes keep the PE busy during the DMA
    # wait so the clock gate is fully open when real work starts.
    warm_ps = sm_pool.tile([P, 4 * P], bf16, tag="sm", name="warmps")
    for _ in range(10):
        nc.tensor.transpose(warm_ps[0:64, 0:P], identity[:, 0:64], identity)

    def transpose_batch(dst, src, b):
        """Transpose 4 (128,128) bf16 tiles src[:, 4b+m, :] into dst[:, 4b+m, :]
        through one 1-bank PSUM tile and a single batched copy."""
        tp = sm_pool.tile([P, 4 * P], bf16, tag="sm")
        for m in range(4):
            nc.tensor.transpose(tp[:, m * P:(m + 1) * P], src[:, 4 * b + m, :], identity)
        nc.vector.tensor_copy(out=dst[:, 4 * b:4 * b + 4, :], in_=tp)

    k_bf = const.tile([P, NT, D], bf16, tag="kbf")
    kT = const.tile([P, NT, P], bf16, tag="kT")  # [d, j, s]
    q_bf = [None] * H
    qb0 = qbp.tile([P, NT, D], bf16, tag="qbf", name="qbf0")
    q_bf[0] = qb0
    for h in range(1, H):
        q_bf[h] = qbp.tile([P, NT, D], bf16, tag="qbf", name=f"qbf{h}")
    qT = [
        qtp.tile([P, NT, P], bf16, tag="qT", name=f"qT{h}") for h in range(H)
    ]  # [d, i, t]
    v_aug = const.tile([P, NT, D + 1], bf16, tag="vaug")

    def k_batch(b):
        nc.vector.tensor_copy(
            out=k_bf[:, 4 * b:4 * b + 4, :], in_=k_sb[:, 4 * b:4 * b + 4, :])
        transpose_batch(kT, k_bf, b)

    def q0_batch(b):
        nc.vector.tensor_copy(
            out=qb0[:, 4 * b:4 * b + 4, :], in_=q0_sb[:, 4 * b:4 * b + 4, :])
        transpose_batch(qT[0], qb0, b)

    def q_cast(h, b):
        nc.gpsimd.tensor_copy(
            out=q_bf[h][:, 4 * b:4 * b + 4, :], in_=q_stg[h][:, 4 * b:4 * b + 4, :])

    def q_tr(h, b):
        transpose_batch(qT[h], q_bf[h], b)

    def v_cast(b):
        nc.gpsimd.tensor_copy(
            out=v_aug[:, 4 * b:4 * b + 4, 0:D], in_=v_sb[:, 4 * b:4 * b + 4, :])

    def v_ones():
        nc.gpsimd.memset(v_aug[:, :, D:D + 1], 1.0)

    # prologue: first k/q0 tail chunks feed head 0's descending start
    k_batch(3)
    q0_batch(3)

    # Static filler schedule: (head, unit_idx) -> list of thunks, emitted
    # just before that unit's S^T matmuls.
    fillers = {}

    def add_f(h, ui, *thunks):
        fillers.setdefault((h, ui), []).extend(thunks)

    add_f(0, 1, lambda: k_batch(2), lambda: q0_batch(2))
    add_f(0, 4, lambda: k_batch(0), lambda: q0_batch(0), lambda: q0_batch(1),
          lambda: v_cast(0), lambda: v_cast(1), v_ones)
    add_f(0, 6, lambda: k_batch(1), lambda: v_cast(2), lambda: v_cast(3))
    add_f(0, 7, lambda: q_cast(1, 0))
    add_f(0, 8, lambda: q_tr(1, 0), lambda: q_cast(1, 1))
    add_f(0, 9, lambda: q_tr(1, 1), lambda: q_cast(1, 2))
    add_f(0, 10, lambda: q_tr(1, 2), lambda: q_cast(1, 3))
    add_f(0, 11, lambda: q_tr(1, 3))
    for hh in (1, 2):
        for b in range(4):
            add_f(hh, 2 + 2 * b, lambda hh=hh, b=b: q_cast(hh + 1, b))
            add_f(hh, 3 + 2 * b, lambda hh=hh, b=b: q_tr(hh + 1, b))

    def emit_chain(eT, h, i):
        """PV accumulation for t-tile i of head h: out_psum (t,129); col 128 is
        the softmax denominator. Normalize and DMA out."""
        pv = sm_pool.tile([P, P + 1], f32, tag="sm")
        for j in range(i + 1):
            c0 = _EOFF[j] + (i - j) * P
            nc.tensor.matmul(
                pv,
                lhsT=eT[:, c0:c0 + P],
                rhs=v_aug[:, j, :],
                start=(j == 0),
                stop=(j == i),
            )
        rec = recp.tile([P, 1], f32, tag="rec")
        nc.vector.reciprocal(rec, pv[:, D:D + 1])
        ot = outp.tile([P, D], f32, tag="outt")
        nc.vector.tensor_scalar_mul(ot, pv[:, 0:D], rec)
        nc.sync.dma_start(o_view[:, i, h, :], ot)

    ready = deque()  # (eT, head, i) PV chains not yet emitted

    def pop_ready(budget, force=False):
        while ready:
            e2, h2, i2 = ready[0]
            size = i2 + 1
            if not force and size > budget and budget < 16:
                break
            ready.popleft()
            emit_chain(e2, h2, i2)
            budget -= size
            if budget <= 0 and not force:
                break

    for h in range(H):
        eT = ep.tile([P, E_COLS], bf16, tag="eT")
        units = _head_units(h)
        for ui, pieces in enumerate(units):
            budget = UNIT
            if ready and ready[0][1] < h:
                budget += 6
            if h == H - 1:
                budget += 8
            pop_ready(budget)
            for f in fillers.get((h, ui), ()):
                f()
            c_lo = min(_EOFF[j] + (ilo - j) * P for (j, ilo, _ln) in pieces)
            n = sum(ln for (_j, _ilo, ln) in pieces)
            stu = st_pool.tile([P, UNIT * P], f32, tag="st")
            for (j, ilo, ln) in pieces:
                pt0 = (_EOFF[j] + (ilo - j) * P - c_lo) // P
                a = 0
                while a < ln:
                    cl = min(4 - (pt0 + a) % 4, ln - a)
                    nc.tensor.matmul(
                        stu[:, (pt0 + a) * P:(pt0 + a + cl) * P],
                        lhsT=kT[:, j, :],
                        rhs=qT[h][:, ilo + a:ilo + a + cl, :],
                        start=True,
                        stop=True,
                    )
                    a += cl
            nc.scalar.activation(
                out=eT[:, c_lo:c_lo + n * P],
                in_=stu[:, 0:n * P],
                func=mybir.ActivationFunctionType.Exp,
                scale=SCALE,
            )
            for (j, ilo, _ln) in pieces:
                if ilo == j:
                    # causal mask on the diagonal tile: keep t_local >= s_local
                    nc.gpsimd.affine_select(
                        out=eT[:, _EOFF[j]:_EOFF[j] + P],
                        in_=eT[:, _EOFF[j]:_EOFF[j] + P],
                        pattern=[[1, P]],
                        compare_op=mybir.AluOpType.is_ge,
                        fill=zfill,
                        base=0,
                        channel_multiplier=-1,
                    )
                    if h > 0 or j < 8:
                        ready.append((eT, h, j))
        if h == 0:
            for i in range(8, NT):
                ready.append((eT, 0, i))
        if h >= 2:
            # chains two heads back must drain before their eT slot recycles
            while ready and ready[0][1] < h - 1:
                e2, h2, i2 = ready.popleft()
                emit_chain(e2, h2, i2)
    pop_ready(0, force=True)


@functools.lru_cache(maxsize=1)
def _build():
    import concourse.tile as tile
    import concourse.mybir as mybir
    from concourse import bacc
    from contextlib import ExitStack

    f32 = mybir.dt.float32
    nc = bacc.Bacc(
        "TRN2",
        target_bir_lowering=False,
        debug=False,
        num_devices=N_CORES,
    )
    q_d = nc.dram_tensor("q", (T, H, D), f32, kind="ExternalInput").ap()
    k_d = nc.dram_tensor("k", (T, D), f32, kind="ExternalInput").ap()
    v_d = nc.dram_tensor("v", (T, D), f32, kind="ExternalInput").ap()
    o_d = nc.dram_tensor("out", (T, H, D), f32, kind="ExternalOutput").ap()

    with tile.TileContext(nc) as tc:
        with ExitStack() as ctx:
            _build_body(tc, nc, q_d, k_d, v_d, o_d, ctx)
    nc.compile()
    return nc


def _in_maps(q, k, v):
    q = np.asarray(q, dtype=np.float32)
    k = np.asarray(k, dtype=np.float32)
    v = np.asarray(v, dtype=np.float32)
    return [
        {
            "q": np.ascontiguousarray(q[:, H * c:H * c + H, :]),
            "k": np.ascontiguousarray(k[:, c, :]),
            "v": np.ascontiguousarray(v[:, c, :]),
        }
        for c in range(N_CORES)
    ]


def kernel(q, k, v, _trace=False):
    from concourse.bass_utils import run_bass_kernel_spmd

    nc = _build()
    res = run_bass_kernel_spmd(
        nc, _in_maps(q, k, v), core_ids=list(range(N_CORES)), trace=_trace
    )
    out = np.empty((T, H_TOTAL, D), dtype=np.float32)
    for c in range(N_CORES):
        out[:, H * c:H * c + H, :] = res.results[c]["out"].reshape(T, H, D)
    if _trace:
        return out, res
    return out


# revision 8
# speedup vs baseline: 1.2057x; 1.1839x over previous
"""GQA causal prefill attention on 8 TRN2 NeuronCores.

Sharding: head-parallel. Core c computes q heads [4c, 4c+4) against kv head c
(n_rep = 4, so the GQA groups align exactly with the shard; no cross-core
communication).

Per-core algorithm (T=2048 tokens, 4 q heads, head_dim 128):
  - Load k, v; build kT (d,s) tiles via PE transpose. v is augmented with a
    ones column -> v_aug (s, 129) in bf16.
  - S^T tiles are packed into 12-tile (3 PSUM bank) units spanning several
    k-tile rows j, so ONE ScalarE exp instruction covers a whole unit. The
    ACT engine is the binding resource (~1.0 ns/col + ~290 ns/inst); packing
    minimizes the per-instruction overhead (~50 exps per core).
  - Causal diagonal masking is done in-place on the bf16 eT buffer by GpSimd
    affine_select (t_local >= s_local), keeping both DVE and ACT off that
    path. q1-3 and v f32->bf16 casts also run on GpSimd.
  - PV with the e^T blocks as the stationary operand and v_aug streaming:
    out_psum (t=128, 129) accumulates over j; column 128 is the softmax
    denominator. Normalize with a per-partition reciprocal multiply (DVE) and
    DMA the (t, d) tile to DRAM.
"""

import sys
import functools

import numpy as np

if "/opt/trn_rl_repo" not in sys.path:
    sys.path.insert(0, "/opt/trn_rl_repo")

T = 2048
H_TOTAL = 32
N_CORES = 8
H = H_TOTAL // N_CORES  # 4 q heads per core
D = 128
P = 128
NT = T // P  # 16 token tiles
SCALE = 0.08838834764831845
UNIT = 12  # tiles per exp unit = 3 PSUM banks

# column offset of s-tile j's slice inside the per-head packed e^T buffer
_EOFF = [0] * (NT + 1)
for _j in range(NT):
    _EOFF[_j + 1] = _EOFF[_j] + (T - P * _j)
E_COLS = _EOFF[NT]  # 17408


def _split_asc(blocks, tail=None):
    """Chop an ascending stream of (j, ilo, ntiles) blocks into units of
    <= UNIT tiles, splitting blocks at tile granularity. `tail` optionally
    forces the sizes of the final units (e.g. [3, 1] for a short drain)."""
    sizes = []
    total = sum(b[2] for b in blocks)
    if tail:
        head = total - sum(tail)
        sizes = [UNIT] * (head // UNIT)
        if head % UNIT:
            sizes.append(head % UNIT)
        sizes += tail
    else:
        sizes = [UNIT] * (total // UNIT)
        if total % UNIT:
            sizes.append(total % UNIT)
    units = []
    cur = []
    cur_n = 0
    si = 0
    for (j, ilo, ln) in blocks:
        a = 0
        while a < ln:
            take = min(sizes[si] - cur_n, ln - a)
            cur.append((j, ilo + a, take))
            cur_n += take
            a += take
            if cur_n == sizes[si]:
                units.append(cur)
                cur = []
                cur_n = 0
                si += 1
    assert not cur
    return units


def _head_units(h):
    """Per head: list of units; unit = list of pieces (j, ilo, ntiles).
    Head 0 starts with descending block-aligned groups (j=15..8) so compute
    begins after only the tail k/q DMA chunks have landed; then j=0..7
    ascending. Heads 1-3 (and head 3 with a short tail) run j ascending."""
    if h == 0:
        units = [
            [(j, j, NT - j) for j in (15, 14, 13, 12)],  # 10 tiles
            [(j, j, NT - j) for j in (11, 10)],  # 11 tiles
            [(9, 9, 7)],
            [(8, 8, 8)],
        ]
        units += _split_asc([(j, j, NT - j) for j in range(8)])
        return units
    tail = [3, 1] if h == H - 1 else None
    return _split_asc([(j, j, NT - j) for j in range(NT)], tail=tail)


def _build_body(tc, nc, q_d, k_d, v_d, o_d, ctx):
    from collections import deque

    import concourse.mybir as mybir
    from concourse.masks import make_identity

    f32 = mybir.dt.float32
    bf16 = mybir.dt.bfloat16

    const = ctx.enter_context(tc.tile_pool(name="const", bufs=1))
    qbp = ctx.enter_context(tc.tile_pool(name="qbf", bufs=4))
    qtp = ctx.enter_context(tc.tile_pool(name="qT", bufs=4))
    ep = ctx.enter_context(tc.tile_pool(name="eT", bufs=2))
    outp = ctx.enter_context(tc.tile_pool(name="outt", bufs=4))
    recp = ctx.enter_context(tc.tile_pool(name="rec", bufs=4))

    # PSUM: two 3-bank S^T units (ping-pong) + two shared 1-bank slots for
    # PV accumulators and transpose staging = exactly 8 banks.
    st_pool = ctx.enter_context(tc.tile_pool(name="st", bufs=2, space="PSUM"))
    sm_pool = ctx.enter_context(tc.tile_pool(name="smp", bufs=2, space="PSUM"))

    # DMA (all on the SP ring, FIFO). k/q0 chunk order matches the compute
    # order: head 0 walks j=15..8 first (needs k/q tail chunks), then 0..7.
    k_view = k_d.rearrange("(j p) d -> p j d", p=P)
    q_view = q_d.rearrange("(i p) h d -> p i h d", p=P)
    o_view = o_d.rearrange("(i p) h d -> p i h d", p=P)
    v_view = v_d.rearrange("(j p) d -> p j d", p=P)

    k_sb = const.tile([P, NT, D], f32, tag="ksb")
    q0_sb = qbp.tile([P, NT, D], f32, tag="qstg", name="q0stg")
    v_sb = const.tile([P, NT, D], f32, tag="vsb")

    def dma_k(b):
        nc.sync.dma_start(k_sb[:, 4 * b:4 * b + 4, :], k_view[:, 4 * b:4 * b + 4, :])

    def dma_q0(b):
        nc.sync.dma_start(
            q0_sb[:, 4 * b:4 * b + 4, :], q_view[:, 4 * b:4 * b + 4, 0, :])

    dma_k(3); dma_q0(3)
    dma_k(2); dma_q0(2)
    dma_k(0); dma_q0(0)
    dma_q0(1); dma_k(1)
    for b in range(2):
        nc.sync.dma_start(v_sb[:, 8 * b:8 * b + 8, :], v_view[:, 8 * b:8 * b + 8, :])
    q_stg = [None] * H
    q_stg[0] = q0_sb
    for h in range(1, H):
        stg = qbp.tile([P, NT, D], f32, tag="qstg", name=f"q{h}stg")
        nc.sync.dma_start(stg, q_view[:, :, h, :])
        q_stg[h] = stg

    identity = const.tile([P, P], bf16, tag="ident")
    make_identity(nc, identity)
    identity_f = const.tile([P, P], f32, tag="identf")
    nc.vector.tensor_copy(out=identity_f, in_=identity)
    zfill = nc.gpsimd.to_reg(0.0)

    # Prewarm the ACT function table so the first real exp doesn't pay the
    # ~1.5us table load on the critical path.
    warm_sb = recp.tile([P, 1], f32, tag="rec", name="warm")
    nc.scalar.activation(
        out=warm_sb, in_=identity[:, 0:1],
        func=mybir.ActivationFunctionType.Exp,
    )

    # Short HAM pre-warm: dummy transposes keep the PE busy during the DMA
    # wait so the clock gate is fully open when real work starts.
    warm_ps = sm_pool.tile([P, 4 * P], bf16, tag="sm", name="warmps")
    for _ in range(10):
        nc.tensor.transpose(warm_ps[0:64, 0:P], identity[:, 0:64], identity)

    def transpose_batch(dst, src, b):
        """Transpose 4 (128,128) f32 tiles src[:, 4b+m, :] into the bf16
        dst[:, 4b+m, :] through one 1-bank PSUM tile; the PSUM->SBUF copy
        does the f32->bf16 conversion, so no separate cast pass is needed."""
        tp = sm_pool.tile([P, 4 * P], f32, tag="sm")
        for m in range(4):
            nc.tensor.transpose(
                tp[:, m * P:(m + 1) * P], src[:, 4 * b + m, :], identity_f)
        nc.vector.tensor_copy(out=dst[:, 4 * b:4 * b + 4, :], in_=tp)

    kT = const.tile([P, NT, P], bf16, tag="kT")  # [d, j, s]
    qT = [
        qtp.tile([P, NT, P], bf16, tag="qT", name=f"qT{h}") for h in range(H)
    ]  # [d, i, t]
    v_aug = const.tile([P, NT, D + 1], bf16, tag="vaug")

    def k_batch(b):
        transpose_batch(kT, k_sb, b)

    def q0_batch(b):
        transpose_batch(qT[0], q0_sb, b)

    def q_tr(h, b):
        transpose_batch(qT[h], q_stg[h], b)

    def v_cast(b):
        nc.vector.tensor_copy(
            out=v_aug[:, 4 * b:4 * b + 4, 0:D], in_=v_sb[:, 4 * b:4 * b + 4, :])

    def v_ones():
        nc.vector.memset(v_aug[:, :, D:D + 1], 1.0)

    # prologue: first k/q0 tail chunks feed head 0's descending start
    k_batch(3)
    q0_batch(3)

    # Static filler schedule: (head, unit_idx) -> list of thunks, emitted
    # just before that unit's S^T matmuls.
    fillers = {}

    def add_f(h, ui, *thunks):
        fillers.setdefault((h, ui), []).extend(thunks)

    add_f(0, 1, lambda: k_batch(2), lambda: q0_batch(2))
    add_f(0, 4, lambda: k_batch(0), lambda: q0_batch(0), lambda: q0_batch(1),
          lambda: v_cast(0), lambda: v_cast(1), v_ones)
    add_f(0, 6, lambda: k_batch(1), lambda: v_cast(2), lambda: v_cast(3))
    for b in range(4):
        add_f(0, 7 + b, lambda b=b: q_tr(1, b))
    for hh in (1, 2):
        for b in range(4):
            add_f(hh, 2 + 2 * b, lambda hh=hh, b=b: q_tr(hh + 1, b))

    def emit_chain(eT, h, i):
        """PV accumulation for t-tile i of head h: out_psum (t,129); col 128 is
        the softmax denominator. Normalize and DMA out."""
        pv = sm_pool.tile([P, P + 1], f32, tag="sm")
        for j in range(i + 1):
            c0 = _EOFF[j] + (i - j) * P
            nc.tensor.matmul(
                pv,
                lhsT=eT[:, c0:c0 + P],
                rhs=v_aug[:, j, :],
                start=(j == 0),
                stop=(j == i),
            )
        rec = recp.tile([P, 1], f32, tag="rec")
        nc.vector.reciprocal(rec, pv[:, D:D + 1])
        ot = outp.tile([P, D], f32, tag="outt")
        nc.vector.tensor_scalar_mul(ot, pv[:, 0:D], rec)
        nc.sync.dma_start(o_view[:, i, h, :], ot)

    ready = deque()  # (eT, head, i) PV chains not yet emitted

    def pop_ready(budget, force=False):
        while ready:
            e2, h2, i2 = ready[0]
            size = i2 + 1
            if not force and size > budget and budget < 16:
                break
            ready.popleft()
            emit_chain(e2, h2, i2)
            budget -= size
            if budget <= 0 and not force:
                break

    for h in range(H):
        eT = ep.tile([P, E_COLS], bf16, tag="eT")
        units = _head_units(h)
        for ui, pieces in enumerate(units):
            for f in fillers.get((h, ui), ()):
                f()
            c_lo = min(_EOFF[j] + (ilo - j) * P for (j, ilo, _ln) in pieces)
            n = sum(ln for (_j, _ilo, ln) in pieces)
            stu = st_pool.tile([P, UNIT * P], f32, tag="st")
            for (j, ilo, ln) in pieces:
                pt0 = (_EOFF[j] + (ilo - j) * P - c_lo) // P
                a = 0
                while a < ln:
                    cl = min(4 - (pt0 + a) % 4, ln - a)
                    nc.tensor.matmul(
                        stu[:, (pt0 + a) * P:(pt0 + a + cl) * P],
                        lhsT=kT[:, j, :],
                        rhs=qT[h][:, ilo + a:ilo + a + cl, :],
                        start=True,
                        stop=True,
                    )
                    a += cl
            nc.scalar.activation(
                out=eT[:, c_lo:c_lo + n * P],
                in_=stu[:, 0:n * P],
                func=mybir.ActivationFunctionType.Exp,
                scale=SCALE,
            )
            for (j, ilo, _ln) in pieces:
                if ilo == j:
                    # causal mask on the diagonal tile: keep t_local >= s_local
                    nc.gpsimd.affine_select(
                        out=eT[:, _EOFF[j]:_EOFF[j] + P],
                        in_=eT[:, _EOFF[j]:_EOFF[j] + P],
                        pattern=[[1, P]],
                        compare_op=mybir.AluOpType.is_ge,
                        fill=zfill,
                        base=0,
                        channel_multiplier=-1,
                    )
                    if h > 0 or j < 8:
                        ready.append((eT, h, j))
            budget = UNIT
            if ready and ready[0][1] < h:
                budget += 6
            if h == H - 1:
                budget += 8
            pop_ready(budget)
        if h == 0:
            for i in range(8, NT):
                ready.append((eT, 0, i))
        if h >= 2:
            # chains two heads back must drain before their eT slot recycles
            while ready and ready[0][1] < h - 1:
                e2, h2, i2 = ready.popleft()
                emit_chain(e2, h2, i2)
    pop_ready(0, force=True)


@functools.lru_cache(maxsize=1)
def _build():
    import concourse.tile as tile
    import concourse.mybir as mybir
    from concourse import bacc
    from contextlib import ExitStack

    f32 = mybir.dt.float32
    nc = bacc.Bacc(
        "TRN2",
        target_bir_lowering=False,
        debug=False,
        num_devices=N_CORES,
    )
    q_d = nc.dram_tensor("q", (T, H, D), f32, kind="ExternalInput").ap()
    k_d = nc.dram_tensor("k", (T, D), f32, kind="ExternalInput").ap()
    v_d = nc.dram_tensor("v", (T, D), f32, kind="ExternalInput").ap()
    o_d = nc.dram_tensor("out", (T, H, D), f32, kind="ExternalOutput").ap()

    with tile.TileContext(nc) as tc:
        with ExitStack() as ctx:
            _build_body(tc, nc, q_d, k_d, v_d, o_d, ctx)
    nc.compile()
    return nc


def _in_maps(q, k, v):
    q = np.asarray(q, dtype=np.float32)
    k = np.asarray(k, dtype=np.float32)
    v = np.asarray(v, dtype=np.float32)
    return [
        {
            "q": np.ascontiguousarray(q[:, H * c:H * c + H, :]),
            "k": np.ascontiguousarray(k[:, c, :]),
            "v": np.ascontiguousarray(v[:, c, :]),
        }
        for c in range(N_CORES)
    ]


def kernel(q, k, v, _trace=False):
    from concourse.bass_utils import run_bass_kernel_spmd

    nc = _build()
    res = run_bass_kernel_spmd(
        nc, _in_maps(q, k, v), core_ids=list(range(N_CORES)), trace=_trace
    )
    out = np.empty((T, H_TOTAL, D), dtype=np.float32)
    for c in range(N_CORES):
        out[:, H * c:H * c + H, :] = res.results[c]["out"].reshape(T, H, D)
    if _trace:
        return out, res
    return out
